# revision 9
# baseline (speedup 1.0000x reference)
"""Multi-head causal attention (B=4, S=2048, D=1024, H=16) on 8 trn2 cores.

Sharding: (batch x head-group) grid -> core c handles batch c//2, heads
[8*(c%2), 8*(c%2)+8).  Each core computes q/k/v projections for its 512
qkv dims, attention for its 8 heads, and a partial output projection.
Host sums the two partial outputs per batch and adds bo.

v3 design (vs v2):
  The v2 trace showed (a) the exp stream started at ~30us (all v MMs
  preceded the first score pair in PE program order), (b) the sync DMA
  queue was the early/mid bottleneck: ~610ns per DMA op regardless of
  size, with 231 ops (75 input chunks + 128 norm-bounce hops + 32
  outputs) serializing input arrival and starving ACT mid-body, and
  (c) ~45us idle at start/tail.  LDWEIGHTS is nearly free in dense
  streams (probe: back-to-back LDW+MM pace ~216ns vs 213ns stream), so
  v3 keeps self-loading matmuls and instead:
  - inputs live in per-class mega-tiles ([P, k, cols] layouts) so the
    whole input load is ~13 large DMAs instead of ~75 chunked ones;
    x + bq on the sync queue, weights + tri on the gpsimd queue, so
    the two streams arrive in parallel;
  - the norm-bounce chain (4 hops x 32 norms) moves to the gpsimd
    queue; outputs keep the sync queue;
  - emission order (== Tile program order == PE priority): qk(m0,sc0)
    -> scores+exp(hp0,qbl0) -> qk(m0,sc1) -> v[0..3] -> attended+norm
    (0,0) -> ..., i.e. the exp stream starts as soon as one qk chunk
    exists, v-groups/qk-chunks/outproj-groups fill PE between
    attention blocks, and outproj groups are spread so the tail ends
    on the shortest chain;
  - outproj PSUM evacuation entirely on DVE (ACT does only exp);
    partial outputs are written bf16 (host sums in f32).
  Numerics identical to v2: bf16 operands, f32 psum, k-bias dropped
  (cancels in softmax), q-bias via DVE tensor_scalar during
  evacuation, v-bias via replicated tile, o-bias on host, softmax
  denominator via a ones-column appended to each head's V tile (M=65
  stationary), causal masking via skipped blocks + additive 0/-1e30
  triangles on diagonal tiles, softmax reciprocal via the DRAM-bounce
  lane-spread trick.
"""

import os
import sys

import numpy as np

sys.path.insert(0, "/opt/trn_rl_repo")

from contextlib import ExitStack

import bass_rust

import concourse.bass as bass
import concourse.mybir as mybir
import concourse.tile as tile

# ---------------------------------------------------------------------------
# Compat shims for bass_rust (new) vs neuronxcc walrus (2026-05-04) skew:
#  1. Tile's epilogue emits EVENT_SEMAPHORE_RANGE_CLEAR (InstISA 176) which
#     this walrus rejects ("ISA wrong length") -> skip it.
#  2. This walrus supports only ONE sync-wait command per instruction; Tile
#     attaches several -> post-pass hoists extra waits onto NOPs inserted
#     just before, on the same engine.
# ---------------------------------------------------------------------------
_MAXW = 1


def _sem_ranges(nums):
    nums = sorted(nums)
    out = []
    start = prev = nums[0]
    for n in nums[1:]:
        if n == prev + 1:
            prev = n
            continue
        out.append(range(start, prev + 1))
        start = prev = n
    out.append(range(start, prev + 1))
    return out


def _install_compat():
    if getattr(bass, "_mha_compat_installed", False):
        return
    bass._mha_compat_installed = True
    from concourse.bass import SemaphoreHandle

    def clear_and_free_semaphores(self, sems):
        if not sems:
            return
        sem_nums = [s.num if isinstance(s, SemaphoreHandle) else s for s in sems]
        for r in _sem_ranges(sem_nums):
            assert self._state.free_isdisjoint(r)
            self.gpsimd.dma_reset(r)
            # skip sem_clear (ISA 176): unsupported by this walrus
        self._state.prepend_free_semaphores(sem_nums)
        for poison_set in self._tile_sem_poison_stack:
            poison_set.update(sem_nums)

    bass.Bass.clear_and_free_semaphores = clear_and_free_semaphores


def _split_sync_waits(nc):
    """Hoist extra sync waits (>_MAXW per instruction) onto NOP carriers."""

    def new_nop(engine):
        binst = nc.engines[engine].isa(
            nc.isa.Opcode.NEURON_ISA_TPB_OPCODE_NOP, {}
        )
        inst = binst.ins
        bb = nc.cur_bb.bb
        assert bb.instructions and bb.instructions[-1] is inst
        bb.instructions.pop()
        return inst

    for func in nc.m.functions:
        for blk in func.blocks:
            snapshot = list(blk.instructions)
            if not any(
                i.sync_info and i.sync_info.on_wait and len(i.sync_info.on_wait) > _MAXW
                for i in snapshot
            ):
                continue
            new = []
            for inst in snapshot:
                si = inst.sync_info
                waits = list(si.on_wait) if si and si.on_wait else []
                if len(waits) > _MAXW:
                    for w in waits[:-_MAXW]:
                        nop = new_nop(inst.engine)
                        nop.sync_info = bass_rust.SyncInfo(on_wait=[w], on_update=[])
                        new.append(nop)
                    upd = list(si.on_update) if si and si.on_update else []
                    inst.sync_info = bass_rust.SyncInfo(
                        on_wait=waits[-_MAXW:], on_update=upd
                    )
                new.append(inst)
            blk.instructions[:] = new

P = 128
S = 2048
D = 1024          # model dim (contraction for qkv / full e for out)
EL = 512          # per-core qkv width (8 heads * 64)
NH = 8            # local heads
DH = 64
NCORES = 8
SCALE = 1.0 / 8.0  # 1/sqrt(DH)
NEG = -1.0e30

ST = S // P       # 16 s-tiles
DT = D // P       # 8 d-tiles
ET = EL // P      # 4 local e-tiles (head pairs)
QB = 4            # q-blocks of 512
EXBUFS = 20

F32 = mybir.dt.float32
BF16 = mybir.dt.bfloat16

_PROGRAM_CACHE = {}


def build_program(mode, split_waits=True):
    """mode: 'causal' (tril mask) or 'full' (no masking)."""
    assert mode in ("causal", "full")
    _install_compat()
    nc = bass.Bass("TRN2", target_bir_lowering=False, debug=False)

    xt_d = nc.dram_tensor("xt", [D, S], BF16, kind="ExternalInput").ap()
    wqt_d = nc.dram_tensor("wqt", [D, EL], BF16, kind="ExternalInput").ap()
    wkt_d = nc.dram_tensor("wkt", [D, EL], BF16, kind="ExternalInput").ap()
    wvt_d = nc.dram_tensor("wvt", [D, EL], BF16, kind="ExternalInput").ap()
    wot_d = nc.dram_tensor("wot", [EL, D], BF16, kind="ExternalInput").ap()
    bq_d = nc.dram_tensor("bq", [EL, 1], F32, kind="ExternalInput").ap()
    bvrep_d = nc.dram_tensor("bvrep", [P, EL], BF16, kind="ExternalInput").ap()
    tri2_d = nc.dram_tensor("tri2", [P, 2 * P], F32, kind="ExternalInput").ap()
    out_d = nc.dram_tensor("out", [S, D], BF16, kind="ExternalOutput").ap()

    causal = mode == "causal"
    Exp = mybir.ActivationFunctionType.Exp

    with ExitStack() as ctx:
        tc = ctx.enter_context(tile.TileContext(nc))
        consts = ctx.enter_context(tc.tile_pool(name="consts", bufs=1))
        wpool = ctx.enter_context(tc.tile_pool(name="w", bufs=1))
        xpool = ctx.enter_context(tc.tile_pool(name="x", bufs=1))
        qkvp = ctx.enter_context(tc.tile_pool(name="qkv", bufs=1))
        attp = ctx.enter_context(tc.tile_pool(name="attsb", bufs=1))
        expp = ctx.enter_context(tc.tile_pool(name="exp", bufs=EXBUFS))
        attup = ctx.enter_context(tc.tile_pool(name="attu", bufs=6))
        smallp = ctx.enter_context(tc.tile_pool(name="small", bufs=6))
        dramp = ctx.enter_context(tc.tile_pool(name="dram", bufs=8, space="DRAM"))
        outp = ctx.enter_context(tc.tile_pool(name="outsb", bufs=3))
        psum = ctx.enter_context(tc.tile_pool(name="ps", bufs=1, space="PSUM"))

        # mega-tiles, flat [P, k*cols] so matmul/LDW operands stay 2D APs;
        # DMA sides use 3D rearranged views of the same memory.
        xt_sb = xpool.tile([P, DT * S], BF16, name="xt")
        wq_sb = wpool.tile([P, DT * EL], BF16, name="wq")
        wk_sb = wpool.tile([P, DT * EL], BF16, name="wk")
        wv_sb = wpool.tile([P, DT * EL], BF16, name="wv")
        wot_sb = wpool.tile([P, ET * D], BF16, name="wo")

        def xsl(k, a, b):
            return xt_sb[:, k * S + a : k * S + b]

        def drearr(ap):
            return ap.rearrange("(k p) c -> p k c", p=P)

        def kview(t, cols):
            return t[:].rearrange("p (k c) -> p k c", c=cols)

        # ---- DMA plan.  sync queue: bq + x (critical path to first scores
        # plus the body's qt/kt chunks; chunk 0/1 per-k so the first qk
        # accumulation chases arrival); gpsimd queue: all weights + tri.
        # Outputs later on sync; norm-bounce hops on gpsimd.
        bq_sb = consts.tile([P, ET], F32)
        nc.sync.dma_start(bq_sb[:], bq_d[:].rearrange("(m p) o -> p (m o)", p=P))
        for k in range(DT):
            nc.sync.dma_start(xsl(k, 0, 512), xt_d[k * P : (k + 1) * P, 0:512])
        nc.gpsimd.dma_start(kview(wq_sb, EL)[:, :, 0:P], drearr(wqt_d[:, 0:P]))
        nc.gpsimd.dma_start(kview(wk_sb, EL)[:, :, 0:P], drearr(wkt_d[:, 0:P]))
        if causal:
            tri2_sb = consts.tile([P, 2 * P], F32)
            nc.gpsimd.dma_start(tri2_sb[:], tri2_d)
        for k in range(DT):
            nc.sync.dma_start(xsl(k, 512, 1024), xt_d[k * P : (k + 1) * P, 512:1024])
        nc.gpsimd.dma_start(kview(wv_sb, EL)[:, :, :], drearr(wvt_d[:, :]))
        bvrep_sb = consts.tile([P, EL], BF16)
        nc.gpsimd.dma_start(bvrep_sb[:], bvrep_d)
        nc.sync.dma_start(
            kview(xt_sb, S)[:, :, 1024:1536], drearr(xt_d[:, 1024:1536])
        )
        nc.sync.dma_start(
            kview(xt_sb, S)[:, :, 1536:2048], drearr(xt_d[:, 1536:2048])
        )
        nc.gpsimd.dma_start(kview(wq_sb, EL)[:, :, P:EL], drearr(wqt_d[:, P:EL]))
        nc.gpsimd.dma_start(kview(wk_sb, EL)[:, :, P:EL], drearr(wkt_d[:, P:EL]))
        nc.gpsimd.dma_start(
            kview(wot_sb, D)[:, :, :], wot_d[:, :].rearrange("(k p) c -> p k c", p=P)
        )

        # ---- qkv outputs + attention result ----
        qt_sb = [qkvp.tile([P, S], BF16, tag=f"qt{m}", name=f"qt{m}") for m in range(ET)]
        kt_sb = [qkvp.tile([P, S], BF16, tag=f"kt{m}", name=f"kt{m}") for m in range(ET)]
        v_sb = [qkvp.tile([P, NH * (DH + 1)], BF16, tag=f"v{st}", name=f"v{st}") for st in range(ST)]
        att_sb = [attp.tile([P, S], BF16, tag=f"att{kt}", name=f"attsb{kt}") for kt in range(ET)]

        def emit_qk_sc(m, sc):
            s0 = sc * 512
            pq = psum.tile([P, 512], F32, tag="pqkv", bufs=2)
            for k in range(DT):
                nc.tensor.matmul(
                    pq[:],
                    wq_sb[:, k * EL + m * P : k * EL + (m + 1) * P],
                    xsl(k, s0, s0 + 512),
                    start=(k == 0),
                    stop=(k == DT - 1),
                )
            nc.vector.tensor_scalar_add(
                qt_sb[m][:, s0 : s0 + 512], pq[:], bq_sb[:, m : m + 1]
            )
            pk = psum.tile([P, 512], F32, tag="pqkv", bufs=2)
            for k in range(DT):
                nc.tensor.matmul(
                    pk[:],
                    wk_sb[:, k * EL + m * P : k * EL + (m + 1) * P],
                    xsl(k, s0, s0 + 512),
                    start=(k == 0),
                    stop=(k == DT - 1),
                )
            nc.vector.tensor_copy(kt_sb[m][:, s0 : s0 + 512], pk[:])

        def emit_v(sts):
            for st in sts:
                pv = psum.tile([P, EL], F32, tag="pqkv", bufs=2)
                for k in range(DT):
                    nc.tensor.matmul(
                        pv[:],
                        xsl(k, st * P, (st + 1) * P),
                        wv_sb[:, k * EL : (k + 1) * EL],
                        start=(k == 0),
                        stop=(k == DT - 1),
                    )
                vdst = v_sb[st][:].rearrange("p (h c) -> p h c", c=DH + 1)
                nc.vector.tensor_add(
                    vdst[:, :, 0:DH],
                    pv[:].rearrange("p (h c) -> p h c", c=DH),
                    bvrep_sb[:].rearrange("p (h c) -> p h c", c=DH),
                )
                nc.vector.memset(vdst[:, :, DH : DH + 1], 1.0)

        def hi_of(qbl):
            return 4 * qbl + 4 if causal else ST

        def alloc_att_ps(hp, qbl):
            return {
                hl: psum.tile([P, 512], F32, tag="att", bufs=2, name=f"attps{hp}{qbl}{hl}")
                for hl in (0, 1)
            }

        def emit_scores_exp(hp, qbl, mks):
            # scores (row-tiled head pairs) + mask + exp
            qb0 = qbl * 512
            exs = []
            for mk in mks:
                k0 = mk * P
                c0 = max(0, k0 - qb0) if causal else 0
                sp = psum.tile([P, 1024], F32, tag="sc", bufs=2)
                for hl in (0, 1):
                    nc.tensor.matmul(
                        sp[:, hl * 512 + c0 : hl * 512 + 512],
                        kt_sb[hp][hl * DH : (hl + 1) * DH, k0 : k0 + P],
                        qt_sb[hp][hl * DH : (hl + 1) * DH, qb0 + c0 : qb0 + 512],
                        start=True,
                        stop=True,
                    )
                spv = sp[:].rearrange("p (l q) -> p l q", q=512)
                if causal and k0 >= qb0:
                    # diagonal tile: 0/-1e30 triangle on both heads
                    nc.vector.tensor_add(
                        spv[:, :, c0 : c0 + P],
                        spv[:, :, c0 : c0 + P],
                        tri2_sb[:].rearrange("p (l q) -> p l q", q=P),
                    )
                ex = expp.tile([P, 1024], BF16, tag="exp", bufs=EXBUFS)
                exv = ex[:].rearrange("p (l q) -> p l q", q=512)
                nc.scalar.activation(
                    exv[:, :, c0:512], spv[:, :, c0:512], Exp, scale=SCALE
                )
                exs.append((mk, exv, c0))
            return exs

        def emit_attended(hp, att_ps, items, mk_hi):
            for mk, exv, c0 in items:
                for hl in (0, 1):
                    h = 2 * hp + hl
                    nc.tensor.matmul(
                        att_ps[hl][0 : DH + 1, c0:512],
                        v_sb[mk][:, h * (DH + 1) : (h + 1) * (DH + 1)],
                        exv[:, hl, c0:512],
                        start=(mk == 0),
                        stop=(mk == mk_hi - 1),
                        skip_group_check=True,
                    )

        def emit_norm(hp, qbl, att_ps):
            qb0 = qbl * 512
            # normalize: evacuate PSUM fast, then recip+broadcast in SBUF.
            # den spread over 32 lanes via a DRAM bounce (SBUF APs cannot
            # repartition or stride-0 broadcast), reciprocal, linearize back,
            # broadcast-read to DH partitions.  Hops ride the gpsimd queue so
            # they never head-of-line-block the sync queue's bulk transfers.
            for hl in (0, 1):
                au = attup.tile([P, 512], BF16, tag="attu")
                nc.vector.tensor_copy(au[0 : DH + 1, :], att_ps[hl][0 : DH + 1, :])
                dend = dramp.tile([1, 512], BF16, tag="dend")
                nc.gpsimd.dma_start(dend[:], au[DH : DH + 1, :])
                denp = smallp.tile([32, 16], BF16, tag="denp")
                nc.gpsimd.dma_start(
                    denp[:], dend[:].rearrange("o (p c) -> (o p) c", c=16)
                )
                with nc.allow_low_precision(reason="softmax denom recip in bf16"):
                    nc.vector.reciprocal(denp[:], denp[:])
                dend2 = dramp.tile([1, 512], BF16, tag="dend2")
                nc.gpsimd.dma_start(
                    dend2[:].rearrange("o (p c) -> (o p) c", c=16), denp[:]
                )
                rep = smallp.tile([DH, 512], BF16, tag="rep")
                nc.gpsimd.dma_start(rep[:], dend2[:].broadcast_to([DH, 512]))
                nc.vector.tensor_mul(
                    att_sb[hp][hl * DH : (hl + 1) * DH, qb0 : qb0 + 512],
                    au[0:DH, :],
                    rep[:],
                )

        def emit_att_norm(hp, qbl, exs):
            att_ps = alloc_att_ps(hp, qbl)
            emit_attended(hp, att_ps, exs, hi_of(qbl))
            emit_norm(hp, qbl, att_ps)

        def emit_outproj(sts):
            for st in sts:
                ot = outp.tile([P, D], BF16, tag="out")
                for eb in range(2):
                    po = psum.tile([P, 512], F32, tag="pqkv", bufs=2, name=f"po{st}_{eb}")
                    for kt in range(ET):
                        nc.tensor.matmul(
                            po[:],
                            att_sb[kt][:, st * P : (st + 1) * P],
                            wot_sb[:, kt * D + eb * 512 : kt * D + eb * 512 + 512],
                            start=(kt == 0),
                            stop=(kt == ET - 1),
                        )
                    nc.vector.tensor_copy(ot[:, eb * 512 : eb * 512 + 512], po[:])
                    nc.sync.dma_start(
                        out_d[st * P : (st + 1) * P, eb * 512 : eb * 512 + 512],
                        ot[:, eb * 512 : eb * 512 + 512],
                    )

        # ---- emission == Tile program order == PE priority.  The exp stream
        # starts as soon as qk(m0,sc0) exists; v-groups/qk-chunks/outproj-
        # groups fill PE between attention blocks; consumers always emitted
        # after their producers (Tile semantics).

        # preload the ACT exp table set (~2.7us) off the critical path: a
        # 1-element exp on a memset scratch right at program start.
        warm = smallp.tile([1, 1], F32, tag="actwarm")
        nc.vector.memset(warm[:], 0.0)
        nc.scalar.activation(warm[:], warm[:], Exp)

        emit_qk_sc(0, 0)
        ex00 = emit_scores_exp(0, 0, range(hi_of(0)))
        emit_qk_sc(0, 1)
        emit_v([0, 1, 2, 3])
        emit_att_norm(0, 0, ex00)
        ex01 = emit_scores_exp(0, 1, range(hi_of(1)))
        emit_qk_sc(0, 2)
        emit_v([4, 5, 6, 7])
        emit_att_norm(0, 1, ex01)
        ex02 = emit_scores_exp(0, 2, range(hi_of(2)))
        emit_qk_sc(0, 3)
        emit_v([8, 9, 10, 11])
        emit_att_norm(0, 2, ex02)
        ex03 = emit_scores_exp(0, 3, range(hi_of(3)))
        emit_qk_sc(1, 0)
        emit_qk_sc(1, 1)
        emit_v([12, 13, 14, 15])
        emit_att_norm(0, 3, ex03)

        def body(hp, qbl, fillers):
            exs = emit_scores_exp(hp, qbl, range(hi_of(qbl)))
            for f in fillers:
                f()
            emit_att_norm(hp, qbl, exs)

        body(1, 0, [lambda: emit_qk_sc(1, 2)])
        body(1, 1, [lambda: emit_qk_sc(1, 3)])
        body(1, 2, [lambda: emit_qk_sc(2, 0)])
        body(1, 3, [lambda: emit_qk_sc(2, 1)])
        body(2, 0, [lambda: emit_qk_sc(2, 2)])
        body(2, 1, [lambda: emit_qk_sc(2, 3)])
        body(2, 2, [lambda: emit_qk_sc(3, 0), lambda: emit_qk_sc(3, 2)])
        body(2, 3, [lambda: emit_qk_sc(3, 1), lambda: emit_qk_sc(3, 3)])
        # hp3 runs its q-blocks longest-first so the kernel tail hangs off
        # the SHORTEST chain (qbl0: 4 exp ops); outproj groups are spread
        # behind the hp3 blocks with reserves covering each norm's
        # DMA-bounce latency so the PE never idles into a HAM re-throttle.
        body(3, 3, [])
        body(3, 2, [lambda: emit_outproj([12, 13])])
        body(3, 1, [lambda: emit_outproj([14, 15, 8, 9])])
        body(3, 0, [lambda: emit_outproj([10, 11, 4, 5])])
        emit_outproj([6, 7])
        emit_outproj([0, 1, 2, 3])

    if split_waits:
        _split_sync_waits(nc)
    return nc


def get_program(mode, split_waits=True):
    key = (mode, split_waits)
    if key not in _PROGRAM_CACHE:
        _PROGRAM_CACHE[key] = build_program(mode, split_waits)
    return _PROGRAM_CACHE[key]


def _detect_mode(mask):
    m = np.asarray(mask)
    if np.array_equal(m != 0, np.tril(np.ones(m.shape, dtype=bool))):
        return "causal"
    if np.all(m != 0):
        return "full"
    raise NotImplementedError("only causal (tril) or all-ones masks supported")


def make_tri2(mode):
    """Additive diagonal-tile mask, doubled along free dim for the two
    heads of a pair: 0 on/above the in-tile diagonal (q >= k, valid),
    -1e30 below (masked)."""
    if mode != "causal":
        return np.zeros((P, 2 * P), dtype=np.float32)
    kk = np.arange(P)[:, None]
    cc = np.arange(P)[None, :]
    tri = np.where(cc >= kk, 0.0, NEG).astype(np.float32)
    return np.concatenate([tri, tri], axis=1)


def make_in_maps(x, Wq, bq, Wk, Wv, bv, Wo, mode):
    bf = mybir.dt.np(BF16)
    x = np.asarray(x, dtype=np.float32)
    B = x.shape[0]
    tri2 = make_tri2(mode)
    xts = [np.ascontiguousarray(x[b].T).astype(bf) for b in range(B)]
    in_maps = []
    for c in range(NCORES):
        b, hg = divmod(c, 2)
        sl = slice(hg * EL, (hg + 1) * EL)
        in_maps.append(
            {
                "xt": xts[b],
                "wqt": np.ascontiguousarray(
                    np.asarray(Wq, np.float32)[sl, :].T
                ).astype(bf),
                "wkt": np.ascontiguousarray(
                    np.asarray(Wk, np.float32)[sl, :].T
                ).astype(bf),
                "wvt": np.ascontiguousarray(
                    np.asarray(Wv, np.float32)[sl, :].T
                ).astype(bf),
                "wot": np.ascontiguousarray(
                    np.asarray(Wo, np.float32)[:, sl].T
                ).astype(bf),
                "bq": np.ascontiguousarray(
                    np.asarray(bq, np.float32)[sl].reshape(EL, 1)
                ),
                "bvrep": np.ascontiguousarray(
                    np.broadcast_to(np.asarray(bv, np.float32)[sl], (P, EL))
                ).astype(bf),
                "tri2": tri2,
            }
        )
    return in_maps


def run(x, mask, Wq, bq, Wk, bk, Wv, bv, Wo, bo, trace=False, **spmd_kwargs):
    """Returns (full_output, BassKernelResults)."""
    from concourse.bass_utils import run_bass_kernel_spmd

    mode = _detect_mode(mask)
    nc = get_program(mode)
    in_maps = make_in_maps(x, Wq, bq, Wk, Wv, bv, Wo, mode)

    res = run_bass_kernel_spmd(
        nc, in_maps, core_ids=list(range(NCORES)), trace=trace, **spmd_kwargs
    )
    B = np.asarray(x).shape[0]
    out = np.empty((B, S, D), dtype=np.float32)
    bo = np.asarray(bo, np.float32)
    for b in range(B):
        out[b] = (
            res.results[2 * b]["out"].astype(np.float32)
            + res.results[2 * b + 1]["out"].astype(np.float32)
            + bo
        )
    return out, res


def kernel(x, mask, Wq, bq, Wk, bk, Wv, bv, Wo, bo):
    out, _ = run(x, mask, Wq, bq, Wk, bk, Wv, bv, Wo, bo)
    return out


# revision 14
# speedup vs baseline: 1.0667x; 1.0667x over previous
"""Multi-head causal attention (B=4, S=2048, D=1024, H=16) on 8 trn2 cores.

Sharding: (batch x head-group) grid -> core c handles batch c//2, heads
[8*(c%2), 8*(c%2)+8).  Each core computes q/k/v projections for its 512
qkv dims, attention for its 8 heads, and a partial output projection.
Host sums the two partial outputs per batch and adds bo.

v3 design (vs v2):
  The v2 trace showed (a) the exp stream started at ~30us (all v MMs
  preceded the first score pair in PE program order), (b) the sync DMA
  queue was the early/mid bottleneck: ~610ns per DMA op regardless of
  size, with 231 ops (75 input chunks + 128 norm-bounce hops + 32
  outputs) serializing input arrival and starving ACT mid-body, and
  (c) ~45us idle at start/tail.  LDWEIGHTS is nearly free in dense
  streams (probe: back-to-back LDW+MM pace ~216ns vs 213ns stream), so
  v3 keeps self-loading matmuls and instead:
  - inputs live in per-class mega-tiles ([P, k, cols] layouts) so the
    whole input load is ~13 large DMAs instead of ~75 chunked ones;
    x + bq on the sync queue, weights + tri on the gpsimd queue, so
    the two streams arrive in parallel;
  - the norm-bounce chain (4 hops x 32 norms) moves to the gpsimd
    queue; outputs keep the sync queue;
  - emission order (== Tile program order == PE priority): qk(m0,sc0)
    -> scores+exp(hp0,qbl0) -> qk(m0,sc1) -> v[0..3] -> attended+norm
    (0,0) -> ..., i.e. the exp stream starts as soon as one qk chunk
    exists, v-groups/qk-chunks/outproj-groups fill PE between
    attention blocks, and outproj groups are spread so the tail ends
    on the shortest chain;
  - outproj PSUM evacuation entirely on DVE (ACT does only exp);
    partial outputs are written bf16 (host sums in f32).
  Numerics identical to v2: bf16 operands, f32 psum, k-bias dropped
  (cancels in softmax), q-bias via DVE tensor_scalar during
  evacuation, v-bias via replicated tile, o-bias on host, softmax
  denominator via a ones-column appended to each head's V tile (M=65
  stationary), causal masking via skipped blocks + additive 0/-1e30
  triangles on diagonal tiles, softmax reciprocal via the DRAM-bounce
  lane-spread trick.
"""

import os
import sys

import numpy as np

sys.path.insert(0, "/opt/trn_rl_repo")

from contextlib import ExitStack

import bass_rust

import concourse.bass as bass
import concourse.mybir as mybir
import concourse.tile as tile

# ---------------------------------------------------------------------------
# Compat shims for bass_rust (new) vs neuronxcc walrus (2026-05-04) skew:
#  1. Tile's epilogue emits EVENT_SEMAPHORE_RANGE_CLEAR (InstISA 176) which
#     this walrus rejects ("ISA wrong length") -> skip it.
#  2. This walrus supports only ONE sync-wait command per instruction; Tile
#     attaches several -> post-pass hoists extra waits onto NOPs inserted
#     just before, on the same engine.
# ---------------------------------------------------------------------------
_MAXW = 1


def _sem_ranges(nums):
    nums = sorted(nums)
    out = []
    start = prev = nums[0]
    for n in nums[1:]:
        if n == prev + 1:
            prev = n
            continue
        out.append(range(start, prev + 1))
        start = prev = n
    out.append(range(start, prev + 1))
    return out


def _install_compat():
    if getattr(bass, "_mha_compat_installed", False):
        return
    bass._mha_compat_installed = True
    from concourse.bass import SemaphoreHandle

    def clear_and_free_semaphores(self, sems):
        if not sems:
            return
        sem_nums = [s.num if isinstance(s, SemaphoreHandle) else s for s in sems]
        for r in _sem_ranges(sem_nums):
            assert self._state.free_isdisjoint(r)
            self.gpsimd.dma_reset(r)
            # skip sem_clear (ISA 176): unsupported by this walrus
        self._state.prepend_free_semaphores(sem_nums)
        for poison_set in self._tile_sem_poison_stack:
            poison_set.update(sem_nums)

    bass.Bass.clear_and_free_semaphores = clear_and_free_semaphores


def _split_sync_waits(nc):
    """Hoist extra sync waits (>_MAXW per instruction) onto NOP carriers."""

    def new_nop(engine):
        binst = nc.engines[engine].isa(
            nc.isa.Opcode.NEURON_ISA_TPB_OPCODE_NOP, {}
        )
        inst = binst.ins
        bb = nc.cur_bb.bb
        assert bb.instructions and bb.instructions[-1] is inst
        bb.instructions.pop()
        return inst

    for func in nc.m.functions:
        for blk in func.blocks:
            snapshot = list(blk.instructions)
            if not any(
                i.sync_info and i.sync_info.on_wait and len(i.sync_info.on_wait) > _MAXW
                for i in snapshot
            ):
                continue
            new = []
            for inst in snapshot:
                si = inst.sync_info
                waits = list(si.on_wait) if si and si.on_wait else []
                if len(waits) > _MAXW:
                    for w in waits[:-_MAXW]:
                        nop = new_nop(inst.engine)
                        nop.sync_info = bass_rust.SyncInfo(on_wait=[w], on_update=[])
                        new.append(nop)
                    upd = list(si.on_update) if si and si.on_update else []
                    inst.sync_info = bass_rust.SyncInfo(
                        on_wait=waits[-_MAXW:], on_update=upd
                    )
                new.append(inst)
            blk.instructions[:] = new

P = 128
S = 2048
D = 1024          # model dim (contraction for qkv / full e for out)
EL = 512          # per-core qkv width (8 heads * 64)
NH = 8            # local heads
DH = 64
NCORES = 8
SCALE = 1.0 / 8.0  # 1/sqrt(DH)
NEG = -1.0e30

ST = S // P       # 16 s-tiles
DT = D // P       # 8 d-tiles
ET = EL // P      # 4 local e-tiles (head pairs)
QB = 4            # q-blocks of 512
EXBUFS = 26

F32 = mybir.dt.float32
BF16 = mybir.dt.bfloat16

_PROGRAM_CACHE = {}


def build_program(mode, split_waits=True):
    """mode: 'causal' (tril mask) or 'full' (no masking)."""
    assert mode in ("causal", "full")
    _install_compat()
    nc = bass.Bass("TRN2", target_bir_lowering=False, debug=False)

    xt_d = nc.dram_tensor("xt", [D, S], BF16, kind="ExternalInput").ap()
    wqt_d = nc.dram_tensor("wqt", [D, EL], BF16, kind="ExternalInput").ap()
    wkt_d = nc.dram_tensor("wkt", [D, EL], BF16, kind="ExternalInput").ap()
    wvt_d = nc.dram_tensor("wvt", [D, EL], BF16, kind="ExternalInput").ap()
    wot_d = nc.dram_tensor("wot", [EL, D], BF16, kind="ExternalInput").ap()
    bq_d = nc.dram_tensor("bq", [EL, 1], F32, kind="ExternalInput").ap()
    bvrep_d = nc.dram_tensor("bvrep", [P, EL], BF16, kind="ExternalInput").ap()
    tri2_d = nc.dram_tensor("tri2", [P, 2 * P], F32, kind="ExternalInput").ap()
    out_d = nc.dram_tensor("out", [S, D], BF16, kind="ExternalOutput").ap()

    causal = mode == "causal"
    Exp = mybir.ActivationFunctionType.Exp

    with ExitStack() as ctx:
        tc = ctx.enter_context(tile.TileContext(nc))
        consts = ctx.enter_context(tc.tile_pool(name="consts", bufs=1))
        wpool = ctx.enter_context(tc.tile_pool(name="w", bufs=1))
        xpool = ctx.enter_context(tc.tile_pool(name="x", bufs=1))
        qkvp = ctx.enter_context(tc.tile_pool(name="qkv", bufs=1))
        attp = ctx.enter_context(tc.tile_pool(name="attsb", bufs=1))
        expp = ctx.enter_context(tc.tile_pool(name="exp", bufs=EXBUFS))
        attup = ctx.enter_context(tc.tile_pool(name="attu", bufs=4))
        smallp = ctx.enter_context(tc.tile_pool(name="small", bufs=6))
        dramp = ctx.enter_context(tc.tile_pool(name="dram", bufs=8, space="DRAM"))
        outp = ctx.enter_context(tc.tile_pool(name="outsb", bufs=3))
        psum = ctx.enter_context(tc.tile_pool(name="ps", bufs=1, space="PSUM"))

        # mega-tiles, flat [P, k*cols] so matmul/LDW operands stay 2D APs;
        # DMA sides use 3D rearranged views of the same memory.
        xt_sb = xpool.tile([P, DT * S], BF16, name="xt")
        wq_sb = wpool.tile([P, DT * EL], BF16, name="wq")
        wk_sb = wpool.tile([P, DT * EL], BF16, name="wk")
        wv_sb = wpool.tile([P, DT * EL], BF16, name="wv")
        wot_sb = wpool.tile([P, ET * D], BF16, name="wo")

        def xsl(k, a, b):
            return xt_sb[:, k * S + a : k * S + b]

        def drearr(ap):
            return ap.rearrange("(k p) c -> p k c", p=P)

        def kview(t, cols):
            return t[:].rearrange("p (k c) -> p k c", c=cols)

        # ---- DMA plan.  sync queue: bq + x (critical path to first scores
        # plus the body's qt/kt chunks; chunk 0/1 per-k so the first qk
        # accumulation chases arrival); gpsimd queue: all weights + tri.
        # Outputs later on sync; norm-bounce hops on gpsimd.
        bq_sb = consts.tile([P, ET], F32)
        nc.sync.dma_start(bq_sb[:], bq_d[:].rearrange("(m p) o -> p (m o)", p=P))
        for k in range(DT):
            nc.sync.dma_start(xsl(k, 0, 512), xt_d[k * P : (k + 1) * P, 0:512])
        nc.gpsimd.dma_start(kview(wq_sb, EL)[:, :, 0:P], drearr(wqt_d[:, 0:P]))
        nc.gpsimd.dma_start(kview(wk_sb, EL)[:, :, 0:P], drearr(wkt_d[:, 0:P]))
        if causal:
            tri2_sb = consts.tile([P, 2 * P], F32)
            nc.gpsimd.dma_start(tri2_sb[:], tri2_d)
        for k in range(DT):
            nc.sync.dma_start(xsl(k, 512, 1024), xt_d[k * P : (k + 1) * P, 512:1024])
        nc.gpsimd.dma_start(kview(wv_sb, EL)[:, :, :], drearr(wvt_d[:, :]))
        bvrep_sb = consts.tile([P, EL], BF16)
        nc.gpsimd.dma_start(bvrep_sb[:], bvrep_d)
        nc.sync.dma_start(
            kview(xt_sb, S)[:, :, 1024:1536], drearr(xt_d[:, 1024:1536])
        )
        nc.sync.dma_start(
            kview(xt_sb, S)[:, :, 1536:2048], drearr(xt_d[:, 1536:2048])
        )
        nc.gpsimd.dma_start(kview(wq_sb, EL)[:, :, P:EL], drearr(wqt_d[:, P:EL]))
        nc.gpsimd.dma_start(kview(wk_sb, EL)[:, :, P:EL], drearr(wkt_d[:, P:EL]))
        nc.gpsimd.dma_start(
            kview(wot_sb, D)[:, :, :], wot_d[:, :].rearrange("(k p) c -> p k c", p=P)
        )

        # ---- qkv outputs + attention result ----
        qt_sb = [qkvp.tile([P, S], BF16, tag=f"qt{m}", name=f"qt{m}") for m in range(ET)]
        kt_sb = [qkvp.tile([P, S], BF16, tag=f"kt{m}", name=f"kt{m}") for m in range(ET)]
        v_sb = [qkvp.tile([P, NH * (DH + 1)], BF16, tag=f"v{st}", name=f"v{st}") for st in range(ST)]
        att_sb = [attp.tile([P, S], BF16, tag=f"att{kt}", name=f"attsb{kt}") for kt in range(ET)]

        def emit_qk_sc(m, sc):
            s0 = sc * 512
            pq = psum.tile([P, 512], F32, tag="pqkv", bufs=2)
            for k in range(DT):
                nc.tensor.matmul(
                    pq[:],
                    wq_sb[:, k * EL + m * P : k * EL + (m + 1) * P],
                    xsl(k, s0, s0 + 512),
                    start=(k == 0),
                    stop=(k == DT - 1),
                )
            nc.vector.tensor_scalar_add(
                qt_sb[m][:, s0 : s0 + 512], pq[:], bq_sb[:, m : m + 1]
            )
            pk = psum.tile([P, 512], F32, tag="pqkv", bufs=2)
            for k in range(DT):
                nc.tensor.matmul(
                    pk[:],
                    wk_sb[:, k * EL + m * P : k * EL + (m + 1) * P],
                    xsl(k, s0, s0 + 512),
                    start=(k == 0),
                    stop=(k == DT - 1),
                )
            nc.vector.tensor_copy(kt_sb[m][:, s0 : s0 + 512], pk[:])

        def emit_v(sts):
            for st in sts:
                pv = psum.tile([P, EL], F32, tag="pqkv", bufs=2)
                for k in range(DT):
                    nc.tensor.matmul(
                        pv[:],
                        xsl(k, st * P, (st + 1) * P),
                        wv_sb[:, k * EL : (k + 1) * EL],
                        start=(k == 0),
                        stop=(k == DT - 1),
                    )
                vdst = v_sb[st][:].rearrange("p (h c) -> p h c", c=DH + 1)
                nc.vector.tensor_add(
                    vdst[:, :, 0:DH],
                    pv[:].rearrange("p (h c) -> p h c", c=DH),
                    bvrep_sb[:].rearrange("p (h c) -> p h c", c=DH),
                )
                nc.vector.memset(vdst[:, :, DH : DH + 1], 1.0)

        def hi_of(qbl):
            return 4 * qbl + 4 if causal else ST

        def alloc_att_ps(hp, qbl):
            return {
                hl: psum.tile([P, 512], F32, tag="att", bufs=2, name=f"attps{hp}{qbl}{hl}")
                for hl in (0, 1)
            }

        def emit_scores_exp(hp, qbl, mks):
            # scores (row-tiled head pairs) + mask + exp
            qb0 = qbl * 512
            exs = []
            for mk in mks:
                k0 = mk * P
                c0 = max(0, k0 - qb0) if causal else 0
                sp = psum.tile([P, 1024], F32, tag="sc", bufs=2)
                for hl in (0, 1):
                    nc.tensor.matmul(
                        sp[:, hl * 512 + c0 : hl * 512 + 512],
                        kt_sb[hp][hl * DH : (hl + 1) * DH, k0 : k0 + P],
                        qt_sb[hp][hl * DH : (hl + 1) * DH, qb0 + c0 : qb0 + 512],
                        start=True,
                        stop=True,
                    )
                spv = sp[:].rearrange("p (l q) -> p l q", q=512)
                if causal and k0 >= qb0:
                    # diagonal tile: 0/-1e30 triangle on both heads
                    nc.vector.tensor_add(
                        spv[:, :, c0 : c0 + P],
                        spv[:, :, c0 : c0 + P],
                        tri2_sb[:].rearrange("p (l q) -> p l q", q=P),
                    )
                ex = expp.tile([P, 1024], BF16, tag="exp", bufs=EXBUFS)
                exv = ex[:].rearrange("p (l q) -> p l q", q=512)
                nc.scalar.activation(
                    exv[:, :, c0:512], spv[:, :, c0:512], Exp, scale=SCALE
                )
                exs.append((mk, exv, c0))
            return exs

        def emit_attended(hp, att_ps, items, mk_hi):
            for mk, exv, c0 in items:
                for hl in (0, 1):
                    h = 2 * hp + hl
                    nc.tensor.matmul(
                        att_ps[hl][0 : DH + 1, c0:512],
                        v_sb[mk][:, h * (DH + 1) : (h + 1) * (DH + 1)],
                        exv[:, hl, c0:512],
                        start=(mk == 0),
                        stop=(mk == mk_hi - 1),
                        skip_group_check=True,
                    )

        def emit_norm(hp, qbl, att_ps):
            qb0 = qbl * 512
            # normalize: evacuate PSUM fast, then recip+broadcast in SBUF.
            # den spread over 32 lanes via a DRAM bounce (SBUF APs cannot
            # repartition or stride-0 broadcast), reciprocal, linearize back,
            # broadcast-read to DH partitions.  Hops ride the gpsimd queue so
            # they never head-of-line-block the sync queue's bulk transfers.
            for hl in (0, 1):
                au = attup.tile([P, 512], BF16, tag="attu")
                nc.vector.tensor_copy(au[0 : DH + 1, :], att_ps[hl][0 : DH + 1, :])
                dend = dramp.tile([1, 512], BF16, tag="dend")
                nc.gpsimd.dma_start(dend[:], au[DH : DH + 1, :])
                denp = smallp.tile([32, 16], BF16, tag="denp")
                nc.gpsimd.dma_start(
                    denp[:], dend[:].rearrange("o (p c) -> (o p) c", c=16)
                )
                with nc.allow_low_precision(reason="softmax denom recip in bf16"):
                    nc.vector.reciprocal(denp[:], denp[:])
                dend2 = dramp.tile([1, 512], BF16, tag="dend2")
                nc.gpsimd.dma_start(
                    dend2[:].rearrange("o (p c) -> (o p) c", c=16), denp[:]
                )
                rep = smallp.tile([DH, 512], BF16, tag="rep")
                nc.gpsimd.dma_start(rep[:], dend2[:].broadcast_to([DH, 512]))
                nc.vector.tensor_mul(
                    att_sb[hp][hl * DH : (hl + 1) * DH, qb0 : qb0 + 512],
                    au[0:DH, :],
                    rep[:],
                )

        def emit_att_norm(hp, qbl, exs):
            att_ps = alloc_att_ps(hp, qbl)
            emit_attended(hp, att_ps, exs, hi_of(qbl))
            emit_norm(hp, qbl, att_ps)

        def emit_outproj(sts):
            for st in sts:
                ot = outp.tile([P, D], BF16, tag="out")
                for eb in range(2):
                    po = psum.tile([P, 512], F32, tag="pqkv", bufs=2, name=f"po{st}_{eb}")
                    for kt in range(ET):
                        nc.tensor.matmul(
                            po[:],
                            att_sb[kt][:, st * P : (st + 1) * P],
                            wot_sb[:, kt * D + eb * 512 : kt * D + eb * 512 + 512],
                            start=(kt == 0),
                            stop=(kt == ET - 1),
                        )
                    nc.vector.tensor_copy(ot[:, eb * 512 : eb * 512 + 512], po[:])
                    nc.sync.dma_start(
                        out_d[st * P : (st + 1) * P, eb * 512 : eb * 512 + 512],
                        ot[:, eb * 512 : eb * 512 + 512],
                    )

        # ---- emission == Tile program order == PE priority.  The exp stream
        # starts as soon as qk(m0,sc0) exists; v-groups/qk-chunks/outproj-
        # groups fill PE between attention blocks; consumers always emitted
        # after their producers (Tile semantics).

        # preload the ACT exp table set (~2.7us) off the critical path: a
        # 1-element exp on a memset scratch right at program start.
        warm = smallp.tile([1, 1], F32, tag="actwarm", bufs=1)
        nc.vector.memset(warm[:], 0.0)
        nc.scalar.activation(warm[:], warm[:], Exp)

        # warm the PE (HAM un-throttles after ~3.4us of sustained busy): a
        # run of dummy N=512 matmuls on memset scratch fills the x-DMA wait
        # so the first real qk/scores run at 2.4GHz instead of 1.2.
        wsrc = smallp.tile([P, P + 512], BF16, tag="pewarm", bufs=1)
        nc.vector.memset(wsrc[:], 0.0)
        for i in range(20):
            pw = psum.tile([P, 512], F32, tag="att", bufs=2)
            nc.tensor.matmul(
                pw[:], wsrc[:, 0:P], wsrc[:, P : P + 512], start=True, stop=True
            )

        # lag-1 pipeline: attended+norm for block i emitted after scores+exp
        # for block i+1, so ACT always has a full block of runway while the
        # PE catches up on attended / v / qk / outproj filler.
        emit_qk_sc(0, 0)
        ex00 = emit_scores_exp(0, 0, range(hi_of(0)))
        emit_qk_sc(0, 1)
        ex01 = emit_scores_exp(0, 1, range(hi_of(1)))
        emit_v([0, 1, 2, 3])
        emit_att_norm(0, 0, ex00)
        emit_qk_sc(0, 2)
        ex02 = emit_scores_exp(0, 2, range(hi_of(2)))
        emit_v([4, 5, 6, 7])
        emit_att_norm(0, 1, ex01)
        emit_qk_sc(0, 3)
        ex03 = emit_scores_exp(0, 3, range(hi_of(3)))
        emit_v([8, 9, 10, 11])
        emit_att_norm(0, 2, ex02)
        emit_qk_sc(1, 0)
        emit_qk_sc(1, 1)
        ex10 = emit_scores_exp(1, 0, range(hi_of(0)))
        emit_v([12, 13, 14, 15])
        emit_att_norm(0, 3, ex03)

        prev = (1, 0, ex10)
        plan = [
            ((1, 1), [lambda: emit_qk_sc(1, 2)]),
            ((1, 2), [lambda: emit_qk_sc(1, 3)]),
            ((1, 3), [lambda: emit_qk_sc(2, 0)]),
            ((2, 0), [lambda: emit_qk_sc(2, 1)]),
            ((2, 1), [lambda: emit_qk_sc(2, 2)]),
            ((2, 2), [lambda: emit_qk_sc(2, 3)]),
            ((2, 3), [lambda: emit_qk_sc(3, 0)]),
            ((3, 0), [lambda: emit_qk_sc(3, 1)]),
            ((3, 1), [lambda: emit_qk_sc(3, 2)]),
            ((3, 2), [lambda: emit_qk_sc(3, 3)]),
            ((3, 3), [lambda: emit_outproj([0, 1, 2, 3])]),
        ]
        for (hp, qbl), fillers in plan:
            exs = emit_scores_exp(hp, qbl, range(hi_of(qbl)))
            for f in fillers:
                f()
            emit_att_norm(*prev)
            prev = (hp, qbl, exs)
        emit_outproj([4, 5, 6, 7])
        emit_att_norm(*prev)  # attended+norm (3,3)
        emit_outproj([8, 9, 10, 11])
        emit_outproj([12, 13, 14, 15])

    if split_waits:
        _split_sync_waits(nc)
    return nc


def get_program(mode, split_waits=True):
    key = (mode, split_waits)
    if key not in _PROGRAM_CACHE:
        _PROGRAM_CACHE[key] = build_program(mode, split_waits)
    return _PROGRAM_CACHE[key]


def _detect_mode(mask):
    m = np.asarray(mask)
    if np.array_equal(m != 0, np.tril(np.ones(m.shape, dtype=bool))):
        return "causal"
    if np.all(m != 0):
        return "full"
    raise NotImplementedError("only causal (tril) or all-ones masks supported")


def make_tri2(mode):
    """Additive diagonal-tile mask, doubled along free dim for the two
    heads of a pair: 0 on/above the in-tile diagonal (q >= k, valid),
    -1e30 below (masked)."""
    if mode != "causal":
        return np.zeros((P, 2 * P), dtype=np.float32)
    kk = np.arange(P)[:, None]
    cc = np.arange(P)[None, :]
    tri = np.where(cc >= kk, 0.0, NEG).astype(np.float32)
    return np.concatenate([tri, tri], axis=1)


def make_in_maps(x, Wq, bq, Wk, Wv, bv, Wo, mode):
    bf = mybir.dt.np(BF16)
    x = np.asarray(x, dtype=np.float32)
    B = x.shape[0]
    tri2 = make_tri2(mode)
    xts = [np.ascontiguousarray(x[b].T).astype(bf) for b in range(B)]
    in_maps = []
    for c in range(NCORES):
        b, hg = divmod(c, 2)
        sl = slice(hg * EL, (hg + 1) * EL)
        in_maps.append(
            {
                "xt": xts[b],
                "wqt": np.ascontiguousarray(
                    np.asarray(Wq, np.float32)[sl, :].T
                ).astype(bf),
                "wkt": np.ascontiguousarray(
                    np.asarray(Wk, np.float32)[sl, :].T
                ).astype(bf),
                "wvt": np.ascontiguousarray(
                    np.asarray(Wv, np.float32)[sl, :].T
                ).astype(bf),
                "wot": np.ascontiguousarray(
                    np.asarray(Wo, np.float32)[:, sl].T
                ).astype(bf),
                "bq": np.ascontiguousarray(
                    np.asarray(bq, np.float32)[sl].reshape(EL, 1)
                ),
                "bvrep": np.ascontiguousarray(
                    np.broadcast_to(np.asarray(bv, np.float32)[sl], (P, EL))
                ).astype(bf),
                "tri2": tri2,
            }
        )
    return in_maps


def run(x, mask, Wq, bq, Wk, bk, Wv, bv, Wo, bo, trace=False, **spmd_kwargs):
    """Returns (full_output, BassKernelResults)."""
    from concourse.bass_utils import run_bass_kernel_spmd

    mode = _detect_mode(mask)
    nc = get_program(mode)
    in_maps = make_in_maps(x, Wq, bq, Wk, Wv, bv, Wo, mode)

    res = run_bass_kernel_spmd(
        nc, in_maps, core_ids=list(range(NCORES)), trace=trace, **spmd_kwargs
    )
    B = np.asarray(x).shape[0]
    out = np.empty((B, S, D), dtype=np.float32)
    bo = np.asarray(bo, np.float32)
    for b in range(B):
        out[b] = (
            res.results[2 * b]["out"].astype(np.float32)
            + res.results[2 * b + 1]["out"].astype(np.float32)
            + bo
        )
    return out, res


def kernel(x, mask, Wq, bq, Wk, bk, Wv, bv, Wo, bo):
    out, _ = run(x, mask, Wq, bq, Wk, bk, Wv, bv, Wo, bo)
    return out
